# revision 42
# baseline (speedup 1.0000x reference)
"""Trainium2 Bass kernel for the LIIF-style guided upsampling MLP (nn_BF_NIR_conv).

Structure (v2 — map-precompute restructure):
`grid_sample(nearest)` at the 4 shifted coords reduces to parity-dependent
integer shifts of the LR grid, so the layer-1 contribution of the sampled
features is a function of the LR cell alone.  We precompute

  F''[cell] = W1f . featc[cell] - (qcy*h) W1y - (qcx*w) W1x     (0 on the halo)
  G'[pixel] = W1g . hr_guide[pixel] + (cy*h) W1y + (cx*w) W1x + b1

on the PE once (14k + 33k rows/core instead of ~200k), and the per-neighbor
layer-1 pre-activation is just  h1pre = F''[window] + G'  — a DVE add of two
bf16 SBUF tiles.  The reference's joint-validity rule (rel uses qc=0 exactly
when the sample is zeroed) makes the halo-zeroed F'' exact at borders — no
fixup tiles needed.  Everything downstream (relu, L2/L3 matmuls, bilateral
softmax combine) runs in bf16 with 1024-pixel chunks.

Bilateral softmax weights: unchanged from v1 (9 shifted 3-channel dot maps,
exp on ACT, selection matmuls, DMA partition-broadcast), kept in fp32.

Sharding: core c handles HR rows [32c, 32c+32) with an 18-row LR halo slice.
"""
import numpy as np

import concourse.bass as bass
import concourse.tile as tile
from concourse import mybir, bacc
from concourse.bass_utils import run_bass_kernel_spmd

F32 = mybir.dt.float32
F16 = mybir.dt.float16
BF16 = mybir.dt.bfloat16
AF = mybir.ActivationFunctionType
ALU = mybir.AluOpType

NCORES = 8
NPF16 = np.float16

_NC = None


def _build_nc():
    global _NC
    if _NC is not None:
        return _NC
    nc = bacc.Bacc("TRN2", target_bir_lowering=False)

    fc0 = nc.dram_tensor("fc0", [128, 2340], F16, kind="ExternalInput")
    fc1 = nc.dram_tensor("fc1", [128, 2340], F16, kind="ExternalInput")
    f3 = nc.dram_tensor("f3", [3, 2340], F32, kind="ExternalInput")
    f3h = nc.dram_tensor("f3h", [3, 2340], F16, kind="ExternalInput")
    guide = nc.dram_tensor("guide", [128, 8192], F16, kind="ExternalInput")
    relc = nc.dram_tensor("relc", [2, 2340], F16, kind="ExternalInput")
    coords = nc.dram_tensor("coords", [2, 8192], F16, kind="ExternalInput")
    w1 = nc.dram_tensor("w1", [128, 768], F16, kind="ExternalInput")
    w1yx = nc.dram_tensor("w1yx", [2, 256], F16, kind="ExternalInput")
    b1t = nc.dram_tensor("b1", [128, 2], F32, kind="ExternalInput")
    w2 = nc.dram_tensor("w2", [128, 256], F16, kind="ExternalInput")
    b2t = nc.dram_tensor("b2", [128, 1], F32, kind="ExternalInput")
    w3 = nc.dram_tensor("w3", [128, 32], F16, kind="ExternalInput")
    b3t = nc.dram_tensor("b3", [128, 1], F32, kind="ExternalInput")
    selS = nc.dram_tensor("selS", [27, 9], F32, kind="ExternalInput")
    selB = nc.dram_tensor("selB", [9, 4], BF16, kind="ExternalInput")
    selE = nc.dram_tensor("selE", [9, 16], BF16, kind="ExternalInput")
    selR = nc.dram_tensor("selR", [128, 32], BF16, kind="ExternalInput")
    # class-grouped output [32ch, cls, k, l] — host de-interleaves
    y = nc.dram_tensor("y", [32, 8192], F16, kind="ExternalOutput")

    with tile.TileContext(nc) as tc, \
         tc.tile_pool(name="const", bufs=1) as constp, \
         tc.tile_pool(name="gpool", bufs=4) as gpool, \
         tc.tile_pool(name="tp", bufs=2) as tp, \
         tc.tile_pool(name="wp", bufs=2) as wp:

        # ---- constants in ----
        s_fc0 = constp.tile([128, 2340], F16)
        s_fc1 = constp.tile([128, 2340], F16)
        s_relc = constp.tile([2, 2340], F16)
        s_w1 = constp.tile([128, 768], F16)
        s_w1yx = constp.tile([2, 256], F16)
        s_b1 = constp.tile([128, 2], F32)
        s_w2 = constp.tile([128, 256], F16)
        s_b2 = constp.tile([128, 1], F32)
        s_w3 = constp.tile([128, 32], F16)
        s_b3 = constp.tile([128, 1], F32)
        s_S = constp.tile([27, 9], F32)
        s_B = constp.tile([9, 4], BF16)
        s_E = constp.tile([9, 16], BF16)
        s_R = constp.tile([128, 32], BF16)
        s_coords = constp.tile([2, 8192], F16)
        # fc halves first so the F'' matmuls can start early
        for half in range(2):
            hs = slice(1170 * half, 1170 * (half + 1))
            nc.sync.dma_start(out=s_fc0[:, hs], in_=fc0[:, hs])
            nc.sync.dma_start(out=s_fc1[:, hs], in_=fc1[:, hs])
        for t_, d_ in [(s_relc, relc),
                       (s_w1, w1), (s_w1yx, w1yx), (s_b1, b1t), (s_w2, w2),
                       (s_b2, b2t), (s_w3, w3), (s_b3, b3t), (s_S, selS),
                       (s_B, selB), (s_E, selE), (s_R, selR),
                       (s_coords, coords)]:
            nc.sync.dma_start(out=t_, in_=d_[:, :])
        s_gds = []
        for cls in range(4):
            s_gd = gpool.tile([128, 2048], F16, tag="gd", name=f"gd{cls}")
            (nc.gpsimd if cls % 2 else nc.sync).dma_start(
                out=s_gd, in_=guide[:, 2048 * cls:2048 * (cls + 1)])
            s_gds.append(s_gd)

        FW = [constp.tile([128, 2340], F16, name=f"FW{b}") for b in range(2)]
        GA = [constp.tile([128, 8192], F16, name=f"GA{b}") for b in range(2)]
        W128 = [constp.tile([128, 2048], BF16, name=f"W128_{c}") for c in range(4)]
        R32 = [constp.tile([32, 2048], BF16, name=f"R32_{c}") for c in range(4)]

        f3r = f3[:, :].rearrange("c (r x) -> c r x", x=130)
        f3hr = f3h[:, :].rearrange("c (r x) -> c r x", x=130)

        # Preamble: T1/T2 window DMAs issue first; PE starts on the F''/G'
        # maps as soon as fc/w1/guide arrive; the bilateral exp/recip compute
        # runs after G' (its W128/R32 results are only needed by predmuls,
        # ~15us into the main loop).
        dma_engs = [nc.gpsimd, nc.sync]
        with tc.tile_pool(name="pipe", bufs=1) as pipe, \
             tc.tile_pool(name="pipe2", bufs=1) as pipe2:
            T1 = pipe.tile([27, 2048], F32, tag="tA")
            e_t = pipe.tile([9, 2048], BF16, tag="tC")
            r_t = pipe.tile([4, 2048], F32, tag="tD")
            r_tb = pipe.tile([4, 2048], BF16, tag="tE")
            T2 = pipe2.tile([27, 2048], F16, tag="tB")
            for g in range(9):
                u, v = divmod(g, 3)
                dma_engs[g % 2].dma_start(
                    out=T2[3 * g:3 * g + 3, :].rearrange("c (r x) -> c r x", x=128),
                    in_=f3hr[0:3, u:u + 16, v:v + 128])
                dma_engs[(g + 1) % 2].dma_start(
                    out=T1[3 * g:3 * g + 3, :].rearrange("c (r x) -> c r x", x=128),
                    in_=f3r[0:3, 1:17, 1:129])
            # ---- F'' map: [128(2blk), 18*130], zero halo, rel folded ----
            # psum tiles [128,1024] (2 banks), 512-wide bank-aligned matmul
            # writes, single evict per tile.
            with tc.tile_pool(name="pF", bufs=2, space="PSUM") as pF:
                for blk in range(2):
                    for ci, (base, wid) in enumerate([(0, 1024), (1024, 1024),
                                                      (2048, 292)]):
                        ps = pF.tile([128, wid], F32, tag=f"fps{wid}")
                        for s0 in range(0, wid, 512):
                            sw = min(512, wid - s0)
                            sl = slice(base + s0, base + s0 + sw)
                            psl = ps[:, s0:s0 + sw]
                            nc.tensor.matmul(psl, s_w1[:, blk * 128:blk * 128 + 128],
                                             s_fc0[:, sl], start=True, stop=False)
                            nc.tensor.matmul(psl,
                                             s_w1[:, 256 + blk * 128:256 + blk * 128 + 128],
                                             s_fc1[:, sl], start=False, stop=False)
                            nc.tensor.matmul(psl, s_w1yx[:, blk * 128:blk * 128 + 128],
                                             s_relc[:, sl], start=False, stop=True)
                        nc.scalar.activation(FW[blk][:, base:base + wid], ps[:, :],
                                             AF.Copy)

            # ---- bilateral weight pipeline (D in fp32, e/r in bf16) ----
            with tc.tile_pool(name="pwt", bufs=1, space="PSUM") as pwt:
                nc.vector.tensor_mul(T1[:, :], T1[:, :], T2[:, :])  # in place
                for ckw in range(4):
                    Dp = pwt.tile([9, 512], F32, tag="wps")
                    nc.tensor.matmul(Dp[:, :], s_S[:, :],
                                     T1[:, 512 * ckw:512 * (ckw + 1)],
                                     start=True, stop=True)
                    nc.scalar.activation(e_t[:, 512 * ckw:512 * (ckw + 1)],
                                         Dp[:, :], AF.Exp)
                for ckw in range(4):
                    sp = pwt.tile([4, 512], F32, tag="wps")
                    nc.tensor.matmul(sp[:, :], s_B[:, :],
                                     e_t[:, 512 * ckw:512 * (ckw + 1)],
                                     start=True, stop=True)
                    nc.vector.reciprocal_approx_fast(
                        out=r_t[:, 512 * ckw:512 * (ckw + 1)], in_=sp[:, :])
                nc.scalar.copy(out=r_tb[:, :], in_=r_t[:, :])
                edram = nc.dram_tensor("edram", [9, 2048], BF16)
                nc.sync.dma_start(out=edram[:, :], in_=e_t[:, :])
                rdram = nc.dram_tensor("rdram", [4, 2048], BF16)
                nc.sync.dma_start(out=rdram[:, :], in_=r_tb[:, :])
                # broadcast rows across partitions (DRAM partition-step-0)
                for cls in range(4):
                    p_, q_ = cls >> 1, cls & 1
                    for j in range(4):
                        a_, b_ = j >> 1, j & 1
                        g = 3 * (p_ + a_) + (q_ + b_)
                        bcast = bass.AP(tensor=edram[:, :].tensor,
                                        offset=g * 2048, ap=[[0, 32], [1, 2048]])
                        dma_engs[(cls * 4 + j) % 2].dma_start(
                            out=W128[cls][32 * j:32 * j + 32, :], in_=bcast)
                    bcast = bass.AP(tensor=rdram[:, :].tensor,
                                    offset=cls * 2048, ap=[[0, 32], [1, 2048]])
                    dma_engs[cls % 2].dma_start(out=R32[cls][:, :], in_=bcast)

            # ---- G' map: [128(2blk), 4cls*2048], coords + b1 folded ----
            with tc.tile_pool(name="pG", bufs=2, space="PSUM") as pG:
                for cls in range(4):
                    for blk in range(2):
                        for hh in range(2):
                            ps = pG.tile([128, 1024], F32, tag="gps")
                            for sub in range(2):
                                s0 = 1024 * hh + 512 * sub
                                sl_a = slice(2048 * cls + s0, 2048 * cls + s0 + 512)
                                psl = ps[:, 512 * sub:512 * (sub + 1)]
                                nc.tensor.matmul(psl,
                                                 s_w1[:, 512 + blk * 128:512 + blk * 128 + 128],
                                                 s_gds[cls][:, s0:s0 + 512],
                                                 start=True, stop=False)
                                nc.tensor.matmul(psl,
                                                 s_w1yx[:, blk * 128:blk * 128 + 128],
                                                 s_coords[:, sl_a],
                                                 start=False, stop=True)
                            osl = slice(2048 * cls + 1024 * hh,
                                        2048 * cls + 1024 * (hh + 1))
                            if blk == 0:
                                nc.scalar.activation(GA[blk][:, osl], ps[:, :],
                                                     AF.Identity, bias=s_b1[:, 0:1])
                            else:
                                nc.vector.tensor_scalar(GA[blk][:, osl], ps[:, :],
                                                        s_b1[:, 1:2], None, ALU.add)

        FWr = [FW[b][:, :].rearrange("c (r x) -> c r x", x=130) for b in range(2)]

        # ---- main per-(cls, ck-pair) pipeline: 1024-pixel units ----
        with tc.tile_pool(name="ph2", bufs=2, space="PSUM") as ph2, \
             tc.tile_pool(name="ppred", bufs=2, space="PSUM") as ppred, \
             tc.tile_pool(name="pout", bufs=1, space="PSUM") as pout:
            for cls in range(4):
                p_, q_ = cls >> 1, cls & 1
                for ckk in range(2):
                    t = [tp.tile([128, 4096], F16, tag=f"t{b}", name=f"t{b}")
                         for b in range(2)]
                    # h1pre = F'' window + G' slice  (adds), then in-place relu.
                    # gpsimd takes 2 adds/unit (tensor_tensor is ~2.2us there
                    # but the engine is otherwise idle); never gpsimd
                    # tensor_scalar (14us+ on Q7).
                    for j in range(4):
                        a_, b_ = j >> 1, j & 1
                        rs = 8 * ckk + p_ + a_
                        cs = q_ + b_
                        for blk in range(2):
                            win = FWr[blk][:, rs:rs + 8, cs:cs + 128]
                            gsl = GA[blk][:, 2048 * cls + 1024 * ckk:
                                          2048 * cls + 1024 * ckk + 1024]
                            dst = t[blk][:, 1024 * j:1024 * (j + 1)]
                            eng = nc.gpsimd if j == 3 else nc.vector
                            eng.tensor_tensor(dst, win, gsl, ALU.add)
                        # relu per (j, blk) region right after its add
                        for blk in range(2):
                            dst = t[blk][:, 1024 * j:1024 * (j + 1)]
                            if j < 2:
                                nc.scalar.activation(dst, dst, AF.Relu)
                            else:
                                nc.vector.tensor_scalar(dst, dst, 0.0, None,
                                                        ALU.max)
                    # layers 2-3; h2 psum [128,1024] tiles, matmuls write
                    # bank-aligned 512 halves (matmul out <= 1 bank; out base
                    # partition must be 0/32/64)
                    csl = slice(2048 * cls + 1024 * ckk, 2048 * cls + 1024 * (ckk + 1))
                    h2sb = tp.tile([128, 4096], F16, tag="h2sb")
                    pw = wp.tile([128, 1024], BF16, tag="pw")
                    for j in range(4):
                        h2ps = ph2.tile([128, 1024], F32, tag="h2ps")
                        for hh in range(2):
                            jsl = slice(1024 * j + 512 * hh, 1024 * j + 512 * (hh + 1))
                            psl = h2ps[:, 512 * hh:512 * (hh + 1)]
                            nc.tensor.matmul(psl, s_w2[:, 0:128],
                                             t[0][:, jsl], start=True, stop=False)
                            nc.tensor.matmul(psl, s_w2[:, 128:256],
                                             t[1][:, jsl], start=False, stop=True)
                        nc.scalar.activation(h2sb[:, 1024 * j:1024 * (j + 1)],
                                             h2ps[:, :], AF.Relu, bias=s_b2[:, 0:1])
                    for jp in range(2):
                        for hh in range(2):
                            hsl = slice(512 * hh, 512 * (hh + 1))
                            pred = ppred.tile([64, 512], F32, tag="pred")
                            for jj in range(2):
                                j = 2 * jp + jj
                                jsl = slice(1024 * j + 512 * hh,
                                            1024 * j + 512 * (hh + 1))
                                nc.tensor.matmul(pred[32 * jj:32 * jj + 32, :],
                                                 s_w3[:, 0:32], h2sb[:, jsl],
                                                 start=True, stop=True)
                            nc.vector.tensor_mul(
                                pw[64 * jp:64 * jp + 64, hsl], pred[:, :],
                                W128[cls][64 * jp:64 * jp + 64,
                                          1024 * ckk + 512 * hh:
                                          1024 * ckk + 512 * (hh + 1)])
                    osb = wp.tile([32, 1024], F16, tag="osb")
                    ops = pout.tile([32, 1024], F32, tag="ops")
                    for hh in range(2):
                        nc.tensor.matmul(ops[:, 512 * hh:512 * (hh + 1)],
                                         s_R[:, 0:32],
                                         pw[:, 512 * hh:512 * (hh + 1)],
                                         start=True, stop=True)
                    nc.vector.tensor_mul(osb[:, :], ops[:, :],
                                         R32[cls][:, 1024 * ckk:1024 * (ckk + 1)])
                    nc.scalar.activation(osb[:, :], osb[:, :], AF.Identity,
                                         bias=s_b3[0:32, 0:1])
                    nc.sync.dma_start(out=y[:, csl], in_=osb[:, :])

    nc.compile()
    _NC = nc
    return nc


def _prep_core(c, feat, lr_guide, hr_guide, W1, b1, W2, b2, W3, b3):
    def pad_slice(img):  # [128, 128, 128] -> [128, 18, 130] zero-padded halo
        out = np.zeros((128, 18, 130), np.float32)
        y0 = 16 * c - 1
        ys, ye = max(y0, 0), min(16 * c + 17, 128)
        out[:, ys - y0:ye - y0, 1:129] = img[:, ys:ye, :]
        return out.reshape(128, 18 * 130)

    fc0_f = pad_slice(lr_guide[0])
    fc1_f = pad_slice(feat[0])
    f3 = np.ascontiguousarray(fc1_f[124:127])

    strip = hr_guide[0][:, 32 * c:32 * c + 32, :]
    g = np.empty((128, 4, 16, 128), np.float32)
    for p in range(2):
        for q in range(2):
            g[:, 2 * p + q] = strip[:, p::2, q::2]

    # rel cell part: -(2i+1-128), -(2jc+1-128); 0 at halo
    y0 = 16 * c - 1
    rr = np.arange(18) + y0
    cc = np.arange(130) - 1
    valid = ((rr >= 0) & (rr < 128))[:, None] & ((cc >= 0) & (cc < 128))[None, :]
    relc = np.zeros((2, 18, 130), np.float32)
    relc[0] = -(2 * rr[:, None] + 1 - 128.0)
    relc[1] = -(2 * cc[None, :] + 1 - 128.0)
    relc[:, ~valid] = 0.0
    relc = relc.reshape(2, 2340)

    # pixel coord part per cls: (I+0.5-128), (J+0.5-128)
    coords = np.zeros((2, 4, 16, 128), np.float32)
    for p in range(2):
        for q in range(2):
            I = 32 * c + 2 * np.arange(16) + p
            J = 2 * np.arange(128) + q
            coords[0, 2 * p + q] = (I + 0.5 - 128.0)[:, None]
            coords[1, 2 * p + q] = (J + 0.5 - 128.0)[None, :]
    coords = coords.reshape(2, 8192)

    w1 = np.stack([W1[0:128], W1[128:256], W1[256:384]], axis=1).reshape(128, 768)
    w1yx = np.ascontiguousarray(W1[384:386])
    b1sb = np.stack([b1[:128], b1[128:]], axis=1)  # [128, 2]
    w2 = np.stack([W2[0:128], W2[128:256]], axis=1).reshape(128, 256)
    b2sb = np.ascontiguousarray(b2[:, None])
    b3sb = np.zeros((128, 1), np.float32)
    b3sb[:32, 0] = b3
    selS = np.zeros((27, 9), np.float32)
    for g9 in range(9):
        for cch in range(3):
            selS[3 * g9 + cch, g9] = 1.0
    selB = np.zeros((9, 4), np.float32)
    for p in range(2):
        for q in range(2):
            for a in range(2):
                for b in range(2):
                    selB[3 * (p + a) + (q + b), 2 * p + q] += 1.0
    selE = np.zeros((9, 16), np.float32)
    for p in range(2):
        for q in range(2):
            for a in range(2):
                for b in range(2):
                    selE[3 * (p + a) + (q + b), 4 * (2 * p + q) + 2 * a + b] = 1.0
    selR = np.zeros((128, 32), np.float32)
    for j in range(4):
        selR[32 * j + np.arange(32), np.arange(32)] = 1.0

    import ml_dtypes
    bf = lambda x: np.ascontiguousarray(x).astype(NPF16)
    bf16 = lambda x: np.ascontiguousarray(x).astype(ml_dtypes.bfloat16)
    return {
        "fc0": bf(fc0_f), "fc1": bf(fc1_f), "f3": f3, "f3h": bf(f3),
        "guide": bf(g.reshape(128, 8192)), "relc": bf(relc), "coords": bf(coords),
        "w1": bf(w1), "w1yx": bf(w1yx), "b1": b1sb,
        "w2": bf(w2), "b2": b2sb, "w3": bf(W3), "b3": b3sb,
        "selS": selS, "selB": bf16(selB), "selE": bf16(selE), "selR": bf16(selR),
    }


def kernel(**inputs):
    feat = np.asarray(inputs["feat"], np.float32)
    lr_guide = np.asarray(inputs["lr_guide"], np.float32)
    hr_guide = np.asarray(inputs["hr_guide"], np.float32)
    W1 = np.asarray(inputs["W1"], np.float32)
    b1 = np.asarray(inputs["b1"], np.float32)
    W2 = np.asarray(inputs["W2"], np.float32)
    b2 = np.asarray(inputs["b2"], np.float32)
    W3 = np.asarray(inputs["W3"], np.float32)
    b3 = np.asarray(inputs["b3"], np.float32)

    nc = _build_nc()
    in_maps = [_prep_core(c, feat, lr_guide, hr_guide, W1, b1, W2, b2, W3, b3)
               for c in range(NCORES)]
    res = run_bass_kernel_spmd(nc, in_maps, core_ids=list(range(NCORES)))
    out = np.zeros((1, 32, 256, 256), np.float32)
    for c in range(NCORES):
        yc = np.asarray(res.results[c]["y"]).astype(np.float32)
        yc = yc.reshape(32, 4, 16, 128)
        strip = out[0, :, 32 * c:32 * c + 32, :]
        for p in range(2):
            for q in range(2):
                strip[:, p::2, q::2] = yc[:, 2 * p + q]
    return out


# revision 43
# speedup vs baseline: 1.1551x; 1.1551x over previous
"""Trainium2 Bass kernel for the LIIF-style guided upsampling MLP (nn_BF_NIR_conv).

Structure (v2 — map-precompute restructure):
`grid_sample(nearest)` at the 4 shifted coords reduces to parity-dependent
integer shifts of the LR grid, so the layer-1 contribution of the sampled
features is a function of the LR cell alone.  We precompute

  F''[cell] = W1f . featc[cell] - (qcy*h) W1y - (qcx*w) W1x     (0 on the halo)
  G'[pixel] = W1g . hr_guide[pixel] + (cy*h) W1y + (cx*w) W1x + b1

on the PE once (14k + 33k rows/core instead of ~200k), and the per-neighbor
layer-1 pre-activation is just  h1pre = F''[window] + G'  — a DVE add of two
bf16 SBUF tiles.  The reference's joint-validity rule (rel uses qc=0 exactly
when the sample is zeroed) makes the halo-zeroed F'' exact at borders — no
fixup tiles needed.  Everything downstream (relu, L2/L3 matmuls, bilateral
softmax combine) runs in bf16 with 1024-pixel chunks.

Bilateral softmax weights: unchanged from v1 (9 shifted 3-channel dot maps,
exp on ACT, selection matmuls, DMA partition-broadcast), kept in fp32.

Sharding: core c handles HR rows [32c, 32c+32) with an 18-row LR halo slice.
"""
import numpy as np

import concourse.bass as bass
import concourse.tile as tile
from concourse import mybir, bacc
from concourse.bass_utils import run_bass_kernel_spmd

F32 = mybir.dt.float32
F16 = mybir.dt.float16
BF16 = mybir.dt.bfloat16
AF = mybir.ActivationFunctionType
ALU = mybir.AluOpType

NCORES = 8
NPF16 = np.float16

_NC = None


def _build_nc():
    global _NC
    if _NC is not None:
        return _NC
    nc = bacc.Bacc("TRN2", target_bir_lowering=False)

    fc0 = nc.dram_tensor("fc0", [128, 2340], F16, kind="ExternalInput")
    fc1 = nc.dram_tensor("fc1", [128, 2340], F16, kind="ExternalInput")
    f3 = nc.dram_tensor("f3", [3, 2340], F32, kind="ExternalInput")
    f3h = nc.dram_tensor("f3h", [3, 2340], F16, kind="ExternalInput")
    guide = nc.dram_tensor("guide", [128, 8192], F16, kind="ExternalInput")
    relc = nc.dram_tensor("relc", [2, 2340], F16, kind="ExternalInput")
    coords = nc.dram_tensor("coords", [2, 8192], F16, kind="ExternalInput")
    w1 = nc.dram_tensor("w1", [128, 768], F16, kind="ExternalInput")
    w1yx = nc.dram_tensor("w1yx", [2, 256], F16, kind="ExternalInput")
    b1t = nc.dram_tensor("b1", [128, 2], F32, kind="ExternalInput")
    w2 = nc.dram_tensor("w2", [128, 256], F16, kind="ExternalInput")
    b2t = nc.dram_tensor("b2", [128, 1], F32, kind="ExternalInput")
    w3 = nc.dram_tensor("w3", [128, 32], F16, kind="ExternalInput")
    b3t = nc.dram_tensor("b3", [128, 1], F32, kind="ExternalInput")
    selS = nc.dram_tensor("selS", [27, 9], F32, kind="ExternalInput")
    selB = nc.dram_tensor("selB", [9, 4], BF16, kind="ExternalInput")
    selE = nc.dram_tensor("selE", [9, 16], BF16, kind="ExternalInput")
    selR = nc.dram_tensor("selR", [128, 32], BF16, kind="ExternalInput")
    # class-grouped output [32ch, cls, k, l] — host de-interleaves
    y = nc.dram_tensor("y", [32, 8192], F16, kind="ExternalOutput")

    with tile.TileContext(nc) as tc, \
         tc.tile_pool(name="const", bufs=1) as constp, \
         tc.tile_pool(name="gpool", bufs=2) as gpool, \
         tc.tile_pool(name="tp", bufs=2) as tp, \
         tc.tile_pool(name="wp", bufs=2) as wp:

        # ---- constants in ----
        s_fc0 = constp.tile([128, 2340], F16)
        s_fc1 = constp.tile([128, 2340], F16)
        s_relc = constp.tile([2, 2340], F16)
        s_w1 = constp.tile([128, 768], F16)
        s_w1yx = constp.tile([2, 256], F16)
        s_b1 = constp.tile([128, 2], F32)
        s_w2 = constp.tile([128, 256], F16)
        s_b2 = constp.tile([128, 1], F32)
        s_w3 = constp.tile([128, 32], F16)
        s_b3 = constp.tile([128, 1], F32)
        s_S = constp.tile([27, 9], F32)
        s_B = constp.tile([9, 4], BF16)
        s_E = constp.tile([9, 16], BF16)
        s_R = constp.tile([128, 32], BF16)
        s_coords = constp.tile([2, 8192], F16)
        for t_, d_ in [(s_fc0, fc0), (s_fc1, fc1), (s_relc, relc),
                       (s_w1, w1), (s_w1yx, w1yx), (s_b1, b1t), (s_w2, w2),
                       (s_b2, b2t), (s_w3, w3), (s_b3, b3t), (s_S, selS),
                       (s_B, selB), (s_E, selE), (s_R, selR),
                       (s_coords, coords)]:
            nc.sync.dma_start(out=t_, in_=d_[:, :])

        FW = [constp.tile([128, 2340], F16, name=f"FW{b}") for b in range(2)]
        GA = [constp.tile([128, 8192], F16, name=f"GA{b}") for b in range(2)]
        W128 = [constp.tile([128, 2048], BF16, name=f"W128_{c}") for c in range(4)]
        R32 = [constp.tile([32, 2048], BF16, name=f"R32_{c}") for c in range(4)]

        f3r = f3[:, :].rearrange("c (r x) -> c r x", x=130)
        f3hr = f3h[:, :].rearrange("c (r x) -> c r x", x=130)

        # Preamble: T1/T2 window DMAs issue first; PE starts on the F''/G'
        # maps as soon as fc/w1/guide arrive; the bilateral exp/recip compute
        # runs after G' (its W128/R32 results are only needed by predmuls,
        # ~15us into the main loop).
        dma_engs = [nc.gpsimd, nc.sync]
        with tc.tile_pool(name="pipe", bufs=1) as pipe, \
             tc.tile_pool(name="pipe2", bufs=1) as pipe2:
            T1 = pipe.tile([27, 2048], F32, tag="tA")
            e_t = pipe.tile([9, 2048], BF16, tag="tC")
            r_t = pipe.tile([4, 2048], F32, tag="tD")
            r_tb = pipe.tile([4, 2048], BF16, tag="tE")
            T2 = pipe2.tile([27, 2048], F32, tag="tB")
            for g in range(9):
                u, v = divmod(g, 3)
                dma_engs[g % 2].dma_start(
                    out=T2[3 * g:3 * g + 3, :].rearrange("c (r x) -> c r x", x=128),
                    in_=f3r[0:3, u:u + 16, v:v + 128])
                dma_engs[(g + 1) % 2].dma_start(
                    out=T1[3 * g:3 * g + 3, :].rearrange("c (r x) -> c r x", x=128),
                    in_=f3r[0:3, 1:17, 1:129])
            # ---- F'' map: [128(2blk), 18*130], zero halo, rel folded ----
            # psum tiles [128,1024] (2 banks), 512-wide bank-aligned matmul
            # writes, single evict per tile.
            with tc.tile_pool(name="pF", bufs=2, space="PSUM") as pF:
                for blk in range(2):
                    for ci, (base, wid) in enumerate([(0, 1024), (1024, 1024),
                                                      (2048, 292)]):
                        ps = pF.tile([128, wid], F32, tag=f"fps{wid}")
                        for s0 in range(0, wid, 512):
                            sw = min(512, wid - s0)
                            sl = slice(base + s0, base + s0 + sw)
                            psl = ps[:, s0:s0 + sw]
                            nc.tensor.matmul(psl, s_w1[:, blk * 128:blk * 128 + 128],
                                             s_fc0[:, sl], start=True, stop=False)
                            nc.tensor.matmul(psl,
                                             s_w1[:, 256 + blk * 128:256 + blk * 128 + 128],
                                             s_fc1[:, sl], start=False, stop=False)
                            nc.tensor.matmul(psl, s_w1yx[:, blk * 128:blk * 128 + 128],
                                             s_relc[:, sl], start=False, stop=True)
                        nc.scalar.activation(FW[blk][:, base:base + wid], ps[:, :],
                                             AF.Copy)

            # ---- bilateral weight pipeline (D in fp32, e/r in bf16) ----
            with tc.tile_pool(name="pwt", bufs=1, space="PSUM") as pwt:
                nc.vector.tensor_mul(T1[:, :], T1[:, :], T2[:, :])  # in place
                for ckw in range(4):
                    Dp = pwt.tile([9, 512], F32, tag="wps")
                    nc.tensor.matmul(Dp[:, :], s_S[:, :],
                                     T1[:, 512 * ckw:512 * (ckw + 1)],
                                     start=True, stop=True)
                    nc.scalar.activation(e_t[:, 512 * ckw:512 * (ckw + 1)],
                                         Dp[:, :], AF.Exp)
                for ckw in range(4):
                    sp = pwt.tile([4, 512], F32, tag="wps")
                    nc.tensor.matmul(sp[:, :], s_B[:, :],
                                     e_t[:, 512 * ckw:512 * (ckw + 1)],
                                     start=True, stop=True)
                    nc.vector.reciprocal_approx_fast(
                        out=r_t[:, 512 * ckw:512 * (ckw + 1)], in_=sp[:, :])
                nc.scalar.copy(out=r_tb[:, :], in_=r_t[:, :])
                edram = nc.dram_tensor("edram", [9, 2048], BF16)
                nc.sync.dma_start(out=edram[:, :], in_=e_t[:, :])
                rdram = nc.dram_tensor("rdram", [4, 2048], BF16)
                nc.sync.dma_start(out=rdram[:, :], in_=r_tb[:, :])
                # broadcast rows across partitions (DRAM partition-step-0)
                for cls in range(4):
                    p_, q_ = cls >> 1, cls & 1
                    for j in range(4):
                        a_, b_ = j >> 1, j & 1
                        g = 3 * (p_ + a_) + (q_ + b_)
                        bcast = bass.AP(tensor=edram[:, :].tensor,
                                        offset=g * 2048, ap=[[0, 32], [1, 2048]])
                        dma_engs[(cls * 4 + j) % 2].dma_start(
                            out=W128[cls][32 * j:32 * j + 32, :], in_=bcast)
                    bcast = bass.AP(tensor=rdram[:, :].tensor,
                                    offset=cls * 2048, ap=[[0, 32], [1, 2048]])
                    dma_engs[cls % 2].dma_start(out=R32[cls][:, :], in_=bcast)

            # ---- G' map: [128(2blk), 4cls*2048], coords + b1 folded ----
            with tc.tile_pool(name="pG", bufs=2, space="PSUM") as pG:
                for cls in range(4):
                    s_gd = gpool.tile([128, 2048], F16, tag="gd")
                    nc.sync.dma_start(out=s_gd,
                                      in_=guide[:, 2048 * cls:2048 * (cls + 1)])
                    for blk in range(2):
                        for hh in range(2):
                            ps = pG.tile([128, 1024], F32, tag="gps")
                            for sub in range(2):
                                s0 = 1024 * hh + 512 * sub
                                sl_a = slice(2048 * cls + s0, 2048 * cls + s0 + 512)
                                psl = ps[:, 512 * sub:512 * (sub + 1)]
                                nc.tensor.matmul(psl,
                                                 s_w1[:, 512 + blk * 128:512 + blk * 128 + 128],
                                                 s_gd[:, s0:s0 + 512],
                                                 start=True, stop=False)
                                nc.tensor.matmul(psl,
                                                 s_w1yx[:, blk * 128:blk * 128 + 128],
                                                 s_coords[:, sl_a],
                                                 start=False, stop=True)
                            osl = slice(2048 * cls + 1024 * hh,
                                        2048 * cls + 1024 * (hh + 1))
                            if blk == 0:
                                nc.scalar.activation(GA[blk][:, osl], ps[:, :],
                                                     AF.Identity, bias=s_b1[:, 0:1])
                            else:
                                nc.vector.tensor_scalar(GA[blk][:, osl], ps[:, :],
                                                        s_b1[:, 1:2], None, ALU.add)

        FWr = [FW[b][:, :].rearrange("c (r x) -> c r x", x=130) for b in range(2)]

        # ---- main per-(cls, ck-pair) pipeline: 1024-pixel units ----
        with tc.tile_pool(name="ph2", bufs=2, space="PSUM") as ph2, \
             tc.tile_pool(name="ppred", bufs=2, space="PSUM") as ppred, \
             tc.tile_pool(name="pout", bufs=1, space="PSUM") as pout:
            for cls in range(4):
                p_, q_ = cls >> 1, cls & 1
                for ckk in range(2):
                    t = [tp.tile([128, 4096], F16, tag=f"t{b}", name=f"t{b}")
                         for b in range(2)]
                    # h1pre = F'' window + G' slice  (adds), then in-place relu.
                    # gpsimd takes 2 adds/unit (tensor_tensor is ~2.2us there
                    # but the engine is otherwise idle); never gpsimd
                    # tensor_scalar (14us+ on Q7).
                    for j in range(4):
                        a_, b_ = j >> 1, j & 1
                        rs = 8 * ckk + p_ + a_
                        cs = q_ + b_
                        for blk in range(2):
                            win = FWr[blk][:, rs:rs + 8, cs:cs + 128]
                            gsl = GA[blk][:, 2048 * cls + 1024 * ckk:
                                          2048 * cls + 1024 * ckk + 1024]
                            dst = t[blk][:, 1024 * j:1024 * (j + 1)]
                            eng = nc.gpsimd if j == 3 else nc.vector
                            eng.tensor_tensor(dst, win, gsl, ALU.add)
                        # relu per (j, blk) region right after its add
                        for blk in range(2):
                            dst = t[blk][:, 1024 * j:1024 * (j + 1)]
                            if j < 2:
                                nc.scalar.activation(dst, dst, AF.Relu)
                            else:
                                nc.vector.tensor_scalar(dst, dst, 0.0, None,
                                                        ALU.max)
                    # layers 2-3; h2 psum [128,1024] tiles, matmuls write
                    # bank-aligned 512 halves (matmul out <= 1 bank; out base
                    # partition must be 0/32/64)
                    csl = slice(2048 * cls + 1024 * ckk, 2048 * cls + 1024 * (ckk + 1))
                    h2sb = tp.tile([128, 4096], F16, tag="h2sb")
                    pw = wp.tile([128, 1024], BF16, tag="pw")
                    for j in range(4):
                        h2ps = ph2.tile([128, 1024], F32, tag="h2ps")
                        for hh in range(2):
                            jsl = slice(1024 * j + 512 * hh, 1024 * j + 512 * (hh + 1))
                            psl = h2ps[:, 512 * hh:512 * (hh + 1)]
                            nc.tensor.matmul(psl, s_w2[:, 0:128],
                                             t[0][:, jsl], start=True, stop=False)
                            nc.tensor.matmul(psl, s_w2[:, 128:256],
                                             t[1][:, jsl], start=False, stop=True)
                        nc.scalar.activation(h2sb[:, 1024 * j:1024 * (j + 1)],
                                             h2ps[:, :], AF.Relu, bias=s_b2[:, 0:1])
                    for jp in range(2):
                        for hh in range(2):
                            hsl = slice(512 * hh, 512 * (hh + 1))
                            pred = ppred.tile([64, 512], F32, tag="pred")
                            for jj in range(2):
                                j = 2 * jp + jj
                                jsl = slice(1024 * j + 512 * hh,
                                            1024 * j + 512 * (hh + 1))
                                nc.tensor.matmul(pred[32 * jj:32 * jj + 32, :],
                                                 s_w3[:, 0:32], h2sb[:, jsl],
                                                 start=True, stop=True)
                            nc.vector.tensor_mul(
                                pw[64 * jp:64 * jp + 64, hsl], pred[:, :],
                                W128[cls][64 * jp:64 * jp + 64,
                                          1024 * ckk + 512 * hh:
                                          1024 * ckk + 512 * (hh + 1)])
                    osb = wp.tile([32, 1024], F16, tag="osb")
                    ops = pout.tile([32, 1024], F32, tag="ops")
                    for hh in range(2):
                        nc.tensor.matmul(ops[:, 512 * hh:512 * (hh + 1)],
                                         s_R[:, 0:32],
                                         pw[:, 512 * hh:512 * (hh + 1)],
                                         start=True, stop=True)
                    nc.vector.tensor_mul(osb[:, :], ops[:, :],
                                         R32[cls][:, 1024 * ckk:1024 * (ckk + 1)])
                    nc.scalar.activation(osb[:, :], osb[:, :], AF.Identity,
                                         bias=s_b3[0:32, 0:1])
                    nc.sync.dma_start(out=y[:, csl], in_=osb[:, :])

    nc.compile()
    _NC = nc
    return nc


def _prep_core(c, feat, lr_guide, hr_guide, W1, b1, W2, b2, W3, b3):
    def pad_slice(img):  # [128, 128, 128] -> [128, 18, 130] zero-padded halo
        out = np.zeros((128, 18, 130), np.float32)
        y0 = 16 * c - 1
        ys, ye = max(y0, 0), min(16 * c + 17, 128)
        out[:, ys - y0:ye - y0, 1:129] = img[:, ys:ye, :]
        return out.reshape(128, 18 * 130)

    fc0_f = pad_slice(lr_guide[0])
    fc1_f = pad_slice(feat[0])
    f3 = np.ascontiguousarray(fc1_f[124:127])

    strip = hr_guide[0][:, 32 * c:32 * c + 32, :]
    g = np.empty((128, 4, 16, 128), np.float32)
    for p in range(2):
        for q in range(2):
            g[:, 2 * p + q] = strip[:, p::2, q::2]

    # rel cell part: -(2i+1-128), -(2jc+1-128); 0 at halo
    y0 = 16 * c - 1
    rr = np.arange(18) + y0
    cc = np.arange(130) - 1
    valid = ((rr >= 0) & (rr < 128))[:, None] & ((cc >= 0) & (cc < 128))[None, :]
    relc = np.zeros((2, 18, 130), np.float32)
    relc[0] = -(2 * rr[:, None] + 1 - 128.0)
    relc[1] = -(2 * cc[None, :] + 1 - 128.0)
    relc[:, ~valid] = 0.0
    relc = relc.reshape(2, 2340)

    # pixel coord part per cls: (I+0.5-128), (J+0.5-128)
    coords = np.zeros((2, 4, 16, 128), np.float32)
    for p in range(2):
        for q in range(2):
            I = 32 * c + 2 * np.arange(16) + p
            J = 2 * np.arange(128) + q
            coords[0, 2 * p + q] = (I + 0.5 - 128.0)[:, None]
            coords[1, 2 * p + q] = (J + 0.5 - 128.0)[None, :]
    coords = coords.reshape(2, 8192)

    w1 = np.stack([W1[0:128], W1[128:256], W1[256:384]], axis=1).reshape(128, 768)
    w1yx = np.ascontiguousarray(W1[384:386])
    b1sb = np.stack([b1[:128], b1[128:]], axis=1)  # [128, 2]
    w2 = np.stack([W2[0:128], W2[128:256]], axis=1).reshape(128, 256)
    b2sb = np.ascontiguousarray(b2[:, None])
    b3sb = np.zeros((128, 1), np.float32)
    b3sb[:32, 0] = b3
    selS = np.zeros((27, 9), np.float32)
    for g9 in range(9):
        for cch in range(3):
            selS[3 * g9 + cch, g9] = 1.0
    selB = np.zeros((9, 4), np.float32)
    for p in range(2):
        for q in range(2):
            for a in range(2):
                for b in range(2):
                    selB[3 * (p + a) + (q + b), 2 * p + q] += 1.0
    selE = np.zeros((9, 16), np.float32)
    for p in range(2):
        for q in range(2):
            for a in range(2):
                for b in range(2):
                    selE[3 * (p + a) + (q + b), 4 * (2 * p + q) + 2 * a + b] = 1.0
    selR = np.zeros((128, 32), np.float32)
    for j in range(4):
        selR[32 * j + np.arange(32), np.arange(32)] = 1.0

    import ml_dtypes
    bf = lambda x: np.ascontiguousarray(x).astype(NPF16)
    bf16 = lambda x: np.ascontiguousarray(x).astype(ml_dtypes.bfloat16)
    return {
        "fc0": bf(fc0_f), "fc1": bf(fc1_f), "f3": f3, "f3h": bf(f3),
        "guide": bf(g.reshape(128, 8192)), "relc": bf(relc), "coords": bf(coords),
        "w1": bf(w1), "w1yx": bf(w1yx), "b1": b1sb,
        "w2": bf(w2), "b2": b2sb, "w3": bf(W3), "b3": b3sb,
        "selS": selS, "selB": bf16(selB), "selE": bf16(selE), "selR": bf16(selR),
    }


def kernel(**inputs):
    feat = np.asarray(inputs["feat"], np.float32)
    lr_guide = np.asarray(inputs["lr_guide"], np.float32)
    hr_guide = np.asarray(inputs["hr_guide"], np.float32)
    W1 = np.asarray(inputs["W1"], np.float32)
    b1 = np.asarray(inputs["b1"], np.float32)
    W2 = np.asarray(inputs["W2"], np.float32)
    b2 = np.asarray(inputs["b2"], np.float32)
    W3 = np.asarray(inputs["W3"], np.float32)
    b3 = np.asarray(inputs["b3"], np.float32)

    nc = _build_nc()
    in_maps = [_prep_core(c, feat, lr_guide, hr_guide, W1, b1, W2, b2, W3, b3)
               for c in range(NCORES)]
    res = run_bass_kernel_spmd(nc, in_maps, core_ids=list(range(NCORES)))
    out = np.zeros((1, 32, 256, 256), np.float32)
    for c in range(NCORES):
        yc = np.asarray(res.results[c]["y"]).astype(np.float32)
        yc = yc.reshape(32, 4, 16, 128)
        strip = out[0, :, 32 * c:32 * c + 32, :]
        for p in range(2):
            for q in range(2):
                strip[:, p::2, q::2] = yc[:, 2 * p + q]
    return out


# revision 45
# speedup vs baseline: 1.1568x; 1.0015x over previous
"""Trainium2 Bass kernel for the LIIF-style guided upsampling MLP (nn_BF_NIR_conv).

Structure (v2 — map-precompute restructure):
`grid_sample(nearest)` at the 4 shifted coords reduces to parity-dependent
integer shifts of the LR grid, so the layer-1 contribution of the sampled
features is a function of the LR cell alone.  We precompute

  F''[cell] = W1f . featc[cell] - (qcy*h) W1y - (qcx*w) W1x     (0 on the halo)
  G'[pixel] = W1g . hr_guide[pixel] + (cy*h) W1y + (cx*w) W1x + b1

on the PE once (14k + 33k rows/core instead of ~200k), and the per-neighbor
layer-1 pre-activation is just  h1pre = F''[window] + G'  — a DVE add of two
bf16 SBUF tiles.  The reference's joint-validity rule (rel uses qc=0 exactly
when the sample is zeroed) makes the halo-zeroed F'' exact at borders — no
fixup tiles needed.  Everything downstream (relu, L2/L3 matmuls, bilateral
softmax combine) runs in bf16 with 1024-pixel chunks.

Bilateral softmax weights: unchanged from v1 (9 shifted 3-channel dot maps,
exp on ACT, selection matmuls, DMA partition-broadcast), kept in fp32.

Sharding: core c handles HR rows [32c, 32c+32) with an 18-row LR halo slice.
"""
import numpy as np

import concourse.bass as bass
import concourse.tile as tile
from concourse import mybir, bacc
from concourse.bass_utils import run_bass_kernel_spmd

F32 = mybir.dt.float32
F16 = mybir.dt.float16
BF16 = mybir.dt.bfloat16
AF = mybir.ActivationFunctionType
ALU = mybir.AluOpType

NCORES = 8
NPF16 = np.float16

_NC = None


def _build_nc():
    global _NC
    if _NC is not None:
        return _NC
    nc = bacc.Bacc("TRN2", target_bir_lowering=False)

    fc0 = nc.dram_tensor("fc0", [128, 2340], F16, kind="ExternalInput")
    fc1 = nc.dram_tensor("fc1", [128, 2340], F16, kind="ExternalInput")
    f3 = nc.dram_tensor("f3", [3, 2340], F32, kind="ExternalInput")
    f3h = nc.dram_tensor("f3h", [3, 2340], F16, kind="ExternalInput")
    guide = nc.dram_tensor("guide", [128, 8192], F16, kind="ExternalInput")
    relc = nc.dram_tensor("relc", [2, 2340], F16, kind="ExternalInput")
    coords = nc.dram_tensor("coords", [2, 8192], F16, kind="ExternalInput")
    w1 = nc.dram_tensor("w1", [128, 768], F16, kind="ExternalInput")
    w1yx = nc.dram_tensor("w1yx", [2, 256], F16, kind="ExternalInput")
    b1t = nc.dram_tensor("b1", [128, 2], F32, kind="ExternalInput")
    w2 = nc.dram_tensor("w2", [128, 256], F16, kind="ExternalInput")
    b2t = nc.dram_tensor("b2", [128, 1], F32, kind="ExternalInput")
    w3 = nc.dram_tensor("w3", [128, 32], F16, kind="ExternalInput")
    b3t = nc.dram_tensor("b3", [128, 1], F32, kind="ExternalInput")
    selS = nc.dram_tensor("selS", [27, 9], F32, kind="ExternalInput")
    selB = nc.dram_tensor("selB", [9, 4], BF16, kind="ExternalInput")
    selE = nc.dram_tensor("selE", [9, 16], BF16, kind="ExternalInput")
    selR = nc.dram_tensor("selR", [128, 32], BF16, kind="ExternalInput")
    # class-grouped output [32ch, cls, k, l] — host de-interleaves
    y = nc.dram_tensor("y", [32, 8192], F16, kind="ExternalOutput")

    with tile.TileContext(nc) as tc, \
         tc.tile_pool(name="const", bufs=1) as constp, \
         tc.tile_pool(name="gpool", bufs=4) as gpool, \
         tc.tile_pool(name="tp", bufs=2) as tp, \
         tc.tile_pool(name="wp", bufs=2) as wp:

        # ---- constants in ----
        s_fc0 = constp.tile([128, 2340], F16)
        s_fc1 = constp.tile([128, 2340], F16)
        s_relc = constp.tile([2, 2340], F16)
        s_w1 = constp.tile([128, 768], F16)
        s_w1yx = constp.tile([2, 256], F16)
        s_b1 = constp.tile([128, 2], F32)
        s_w2 = constp.tile([128, 256], F16)
        s_b2 = constp.tile([128, 1], F32)
        s_w3 = constp.tile([128, 32], F16)
        s_b3 = constp.tile([128, 1], F32)
        s_S = constp.tile([27, 9], F32)
        s_B = constp.tile([9, 4], BF16)
        s_E = constp.tile([9, 16], BF16)
        s_R = constp.tile([128, 32], BF16)
        s_coords = constp.tile([2, 8192], F16)
        for t_, d_ in [(s_fc0, fc0), (s_fc1, fc1), (s_relc, relc),
                       (s_w1, w1), (s_w1yx, w1yx), (s_b1, b1t), (s_w2, w2),
                       (s_b2, b2t), (s_w3, w3), (s_b3, b3t), (s_S, selS),
                       (s_B, selB), (s_E, selE), (s_R, selR),
                       (s_coords, coords)]:
            nc.sync.dma_start(out=t_, in_=d_[:, :])
        s_gds = []
        for cls in range(4):
            s_gd_ = gpool.tile([128, 2048], F16, tag="gd", name=f"gd{cls}")
            (nc.gpsimd if cls % 2 else nc.sync).dma_start(
                out=s_gd_, in_=guide[:, 2048 * cls:2048 * (cls + 1)])
            s_gds.append(s_gd_)

        FW = [constp.tile([128, 2340], F16, name=f"FW{b}") for b in range(2)]
        GA = [constp.tile([128, 8192], F16, name=f"GA{b}") for b in range(2)]
        W128 = [constp.tile([128, 2048], BF16, name=f"W128_{c}") for c in range(4)]
        R32 = [constp.tile([32, 2048], BF16, name=f"R32_{c}") for c in range(4)]

        f3r = f3[:, :].rearrange("c (r x) -> c r x", x=130)
        f3hr = f3h[:, :].rearrange("c (r x) -> c r x", x=130)

        # Preamble: T1/T2 window DMAs issue first; PE starts on the F''/G'
        # maps as soon as fc/w1/guide arrive; the bilateral exp/recip compute
        # runs after G' (its W128/R32 results are only needed by predmuls,
        # ~15us into the main loop).
        dma_engs = [nc.gpsimd, nc.sync]
        with tc.tile_pool(name="pipe", bufs=1) as pipe, \
             tc.tile_pool(name="pipe2", bufs=1) as pipe2:
            T1 = pipe.tile([27, 2048], F32, tag="tA")
            e_t = pipe.tile([9, 2048], BF16, tag="tC")
            r_t = pipe.tile([4, 2048], F32, tag="tD")
            r_tb = pipe.tile([4, 2048], BF16, tag="tE")
            T2 = pipe2.tile([27, 2048], F16, tag="tB")
            for g in range(9):
                u, v = divmod(g, 3)
                dma_engs[g % 2].dma_start(
                    out=T2[3 * g:3 * g + 3, :].rearrange("c (r x) -> c r x", x=128),
                    in_=f3hr[0:3, u:u + 16, v:v + 128])
                dma_engs[(g + 1) % 2].dma_start(
                    out=T1[3 * g:3 * g + 3, :].rearrange("c (r x) -> c r x", x=128),
                    in_=f3r[0:3, 1:17, 1:129])
            # ---- F'' map: [128(2blk), 18*130], zero halo, rel folded ----
            # psum tiles [128,1024] (2 banks), 512-wide bank-aligned matmul
            # writes, single evict per tile.
            with tc.tile_pool(name="pF", bufs=2, space="PSUM") as pF:
                for blk in range(2):
                    for ci, (base, wid) in enumerate([(0, 1024), (1024, 1024),
                                                      (2048, 292)]):
                        ps = pF.tile([128, wid], F32, tag=f"fps{wid}")
                        for s0 in range(0, wid, 512):
                            sw = min(512, wid - s0)
                            sl = slice(base + s0, base + s0 + sw)
                            psl = ps[:, s0:s0 + sw]
                            nc.tensor.matmul(psl, s_w1[:, blk * 128:blk * 128 + 128],
                                             s_fc0[:, sl], start=True, stop=False)
                            nc.tensor.matmul(psl,
                                             s_w1[:, 256 + blk * 128:256 + blk * 128 + 128],
                                             s_fc1[:, sl], start=False, stop=False)
                            nc.tensor.matmul(psl, s_w1yx[:, blk * 128:blk * 128 + 128],
                                             s_relc[:, sl], start=False, stop=True)
                        nc.scalar.activation(FW[blk][:, base:base + wid], ps[:, :],
                                             AF.Copy)

            # ---- bilateral weight pipeline (D in fp32, e/r in bf16) ----
            with tc.tile_pool(name="pwt", bufs=1, space="PSUM") as pwt:
                nc.vector.tensor_mul(T1[:, :], T1[:, :], T2[:, :])  # in place
                for ckw in range(4):
                    Dp = pwt.tile([9, 512], F32, tag="wps")
                    nc.tensor.matmul(Dp[:, :], s_S[:, :],
                                     T1[:, 512 * ckw:512 * (ckw + 1)],
                                     start=True, stop=True)
                    nc.scalar.activation(e_t[:, 512 * ckw:512 * (ckw + 1)],
                                         Dp[:, :], AF.Exp)
                for ckw in range(4):
                    sp = pwt.tile([4, 512], F32, tag="wps")
                    nc.tensor.matmul(sp[:, :], s_B[:, :],
                                     e_t[:, 512 * ckw:512 * (ckw + 1)],
                                     start=True, stop=True)
                    nc.vector.reciprocal_approx_fast(
                        out=r_t[:, 512 * ckw:512 * (ckw + 1)], in_=sp[:, :])
                nc.scalar.copy(out=r_tb[:, :], in_=r_t[:, :])
                edram = nc.dram_tensor("edram", [9, 2048], BF16)
                nc.sync.dma_start(out=edram[:, :], in_=e_t[:, :])
                rdram = nc.dram_tensor("rdram", [4, 2048], BF16)
                nc.sync.dma_start(out=rdram[:, :], in_=r_tb[:, :])
                # broadcast rows across partitions (DRAM partition-step-0)
                for cls in range(4):
                    p_, q_ = cls >> 1, cls & 1
                    for j in range(4):
                        a_, b_ = j >> 1, j & 1
                        g = 3 * (p_ + a_) + (q_ + b_)
                        bcast = bass.AP(tensor=edram[:, :].tensor,
                                        offset=g * 2048, ap=[[0, 32], [1, 2048]])
                        dma_engs[(cls * 4 + j) % 2].dma_start(
                            out=W128[cls][32 * j:32 * j + 32, :], in_=bcast)
                    bcast = bass.AP(tensor=rdram[:, :].tensor,
                                    offset=cls * 2048, ap=[[0, 32], [1, 2048]])
                    dma_engs[cls % 2].dma_start(out=R32[cls][:, :], in_=bcast)

            # ---- G' map: [128(2blk), 4cls*2048], coords + b1 folded ----
            with tc.tile_pool(name="pG", bufs=2, space="PSUM") as pG:
                for cls in range(4):
                    s_gd = s_gds[cls]
                    for blk in range(2):
                        for hh in range(2):
                            ps = pG.tile([128, 1024], F32, tag="gps")
                            for sub in range(2):
                                s0 = 1024 * hh + 512 * sub
                                sl_a = slice(2048 * cls + s0, 2048 * cls + s0 + 512)
                                psl = ps[:, 512 * sub:512 * (sub + 1)]
                                nc.tensor.matmul(psl,
                                                 s_w1[:, 512 + blk * 128:512 + blk * 128 + 128],
                                                 s_gd[:, s0:s0 + 512],
                                                 start=True, stop=False)
                                nc.tensor.matmul(psl,
                                                 s_w1yx[:, blk * 128:blk * 128 + 128],
                                                 s_coords[:, sl_a],
                                                 start=False, stop=True)
                            osl = slice(2048 * cls + 1024 * hh,
                                        2048 * cls + 1024 * (hh + 1))
                            if blk == 0:
                                nc.scalar.activation(GA[blk][:, osl], ps[:, :],
                                                     AF.Identity, bias=s_b1[:, 0:1])
                            else:
                                nc.vector.tensor_scalar(GA[blk][:, osl], ps[:, :],
                                                        s_b1[:, 1:2], None, ALU.add)

        FWr = [FW[b][:, :].rearrange("c (r x) -> c r x", x=130) for b in range(2)]

        # ---- main per-(cls, ck-pair) pipeline: 1024-pixel units ----
        with tc.tile_pool(name="ph2", bufs=2, space="PSUM") as ph2, \
             tc.tile_pool(name="ppred", bufs=2, space="PSUM") as ppred, \
             tc.tile_pool(name="pout", bufs=1, space="PSUM") as pout:
            for cls in range(4):
                p_, q_ = cls >> 1, cls & 1
                for ckk in range(2):
                    t = [tp.tile([128, 4096], F16, tag=f"t{b}", name=f"t{b}")
                         for b in range(2)]
                    # h1pre = F'' window + G' slice  (adds), then in-place relu.
                    # gpsimd takes 2 adds/unit (tensor_tensor is ~2.2us there
                    # but the engine is otherwise idle); never gpsimd
                    # tensor_scalar (14us+ on Q7).
                    for j in range(4):
                        a_, b_ = j >> 1, j & 1
                        rs = 8 * ckk + p_ + a_
                        cs = q_ + b_
                        for blk in range(2):
                            win = FWr[blk][:, rs:rs + 8, cs:cs + 128]
                            gsl = GA[blk][:, 2048 * cls + 1024 * ckk:
                                          2048 * cls + 1024 * ckk + 1024]
                            dst = t[blk][:, 1024 * j:1024 * (j + 1)]
                            eng = nc.gpsimd if j == 3 else nc.vector
                            eng.tensor_tensor(dst, win, gsl, ALU.add)
                        # relu per (j, blk) region right after its add
                        for blk in range(2):
                            dst = t[blk][:, 1024 * j:1024 * (j + 1)]
                            if j < 2:
                                nc.scalar.activation(dst, dst, AF.Relu)
                            else:
                                nc.vector.tensor_scalar(dst, dst, 0.0, None,
                                                        ALU.max)
                    # layers 2-3; h2 psum [128,1024] tiles, matmuls write
                    # bank-aligned 512 halves (matmul out <= 1 bank; out base
                    # partition must be 0/32/64)
                    csl = slice(2048 * cls + 1024 * ckk, 2048 * cls + 1024 * (ckk + 1))
                    h2sb = tp.tile([128, 4096], F16, tag="h2sb")
                    pw = wp.tile([128, 1024], BF16, tag="pw")
                    for j in range(4):
                        h2ps = ph2.tile([128, 1024], F32, tag="h2ps")
                        for hh in range(2):
                            jsl = slice(1024 * j + 512 * hh, 1024 * j + 512 * (hh + 1))
                            psl = h2ps[:, 512 * hh:512 * (hh + 1)]
                            nc.tensor.matmul(psl, s_w2[:, 0:128],
                                             t[0][:, jsl], start=True, stop=False)
                            nc.tensor.matmul(psl, s_w2[:, 128:256],
                                             t[1][:, jsl], start=False, stop=True)
                        nc.scalar.activation(h2sb[:, 1024 * j:1024 * (j + 1)],
                                             h2ps[:, :], AF.Relu, bias=s_b2[:, 0:1])
                    for jp in range(2):
                        for hh in range(2):
                            hsl = slice(512 * hh, 512 * (hh + 1))
                            pred = ppred.tile([64, 512], F32, tag="pred")
                            for jj in range(2):
                                j = 2 * jp + jj
                                jsl = slice(1024 * j + 512 * hh,
                                            1024 * j + 512 * (hh + 1))
                                nc.tensor.matmul(pred[32 * jj:32 * jj + 32, :],
                                                 s_w3[:, 0:32], h2sb[:, jsl],
                                                 start=True, stop=True)
                            nc.vector.tensor_mul(
                                pw[64 * jp:64 * jp + 64, hsl], pred[:, :],
                                W128[cls][64 * jp:64 * jp + 64,
                                          1024 * ckk + 512 * hh:
                                          1024 * ckk + 512 * (hh + 1)])
                    osb = wp.tile([32, 1024], F16, tag="osb")
                    ops = pout.tile([32, 1024], F32, tag="ops")
                    for hh in range(2):
                        nc.tensor.matmul(ops[:, 512 * hh:512 * (hh + 1)],
                                         s_R[:, 0:32],
                                         pw[:, 512 * hh:512 * (hh + 1)],
                                         start=True, stop=True)
                    nc.vector.tensor_mul(osb[:, :], ops[:, :],
                                         R32[cls][:, 1024 * ckk:1024 * (ckk + 1)])
                    nc.scalar.activation(osb[:, :], osb[:, :], AF.Identity,
                                         bias=s_b3[0:32, 0:1])
                    nc.sync.dma_start(out=y[:, csl], in_=osb[:, :])

    nc.compile()
    _NC = nc
    return nc


def _prep_core(c, feat, lr_guide, hr_guide, W1, b1, W2, b2, W3, b3):
    def pad_slice(img):  # [128, 128, 128] -> [128, 18, 130] zero-padded halo
        out = np.zeros((128, 18, 130), np.float32)
        y0 = 16 * c - 1
        ys, ye = max(y0, 0), min(16 * c + 17, 128)
        out[:, ys - y0:ye - y0, 1:129] = img[:, ys:ye, :]
        return out.reshape(128, 18 * 130)

    fc0_f = pad_slice(lr_guide[0])
    fc1_f = pad_slice(feat[0])
    f3 = np.ascontiguousarray(fc1_f[124:127])

    strip = hr_guide[0][:, 32 * c:32 * c + 32, :]
    g = np.empty((128, 4, 16, 128), np.float32)
    for p in range(2):
        for q in range(2):
            g[:, 2 * p + q] = strip[:, p::2, q::2]

    # rel cell part: -(2i+1-128), -(2jc+1-128); 0 at halo
    y0 = 16 * c - 1
    rr = np.arange(18) + y0
    cc = np.arange(130) - 1
    valid = ((rr >= 0) & (rr < 128))[:, None] & ((cc >= 0) & (cc < 128))[None, :]
    relc = np.zeros((2, 18, 130), np.float32)
    relc[0] = -(2 * rr[:, None] + 1 - 128.0)
    relc[1] = -(2 * cc[None, :] + 1 - 128.0)
    relc[:, ~valid] = 0.0
    relc = relc.reshape(2, 2340)

    # pixel coord part per cls: (I+0.5-128), (J+0.5-128)
    coords = np.zeros((2, 4, 16, 128), np.float32)
    for p in range(2):
        for q in range(2):
            I = 32 * c + 2 * np.arange(16) + p
            J = 2 * np.arange(128) + q
            coords[0, 2 * p + q] = (I + 0.5 - 128.0)[:, None]
            coords[1, 2 * p + q] = (J + 0.5 - 128.0)[None, :]
    coords = coords.reshape(2, 8192)

    w1 = np.stack([W1[0:128], W1[128:256], W1[256:384]], axis=1).reshape(128, 768)
    w1yx = np.ascontiguousarray(W1[384:386])
    b1sb = np.stack([b1[:128], b1[128:]], axis=1)  # [128, 2]
    w2 = np.stack([W2[0:128], W2[128:256]], axis=1).reshape(128, 256)
    b2sb = np.ascontiguousarray(b2[:, None])
    b3sb = np.zeros((128, 1), np.float32)
    b3sb[:32, 0] = b3
    selS = np.zeros((27, 9), np.float32)
    for g9 in range(9):
        for cch in range(3):
            selS[3 * g9 + cch, g9] = 1.0
    selB = np.zeros((9, 4), np.float32)
    for p in range(2):
        for q in range(2):
            for a in range(2):
                for b in range(2):
                    selB[3 * (p + a) + (q + b), 2 * p + q] += 1.0
    selE = np.zeros((9, 16), np.float32)
    for p in range(2):
        for q in range(2):
            for a in range(2):
                for b in range(2):
                    selE[3 * (p + a) + (q + b), 4 * (2 * p + q) + 2 * a + b] = 1.0
    selR = np.zeros((128, 32), np.float32)
    for j in range(4):
        selR[32 * j + np.arange(32), np.arange(32)] = 1.0

    import ml_dtypes
    bf = lambda x: np.ascontiguousarray(x).astype(NPF16)
    bf16 = lambda x: np.ascontiguousarray(x).astype(ml_dtypes.bfloat16)
    return {
        "fc0": bf(fc0_f), "fc1": bf(fc1_f), "f3": f3, "f3h": bf(f3),
        "guide": bf(g.reshape(128, 8192)), "relc": bf(relc), "coords": bf(coords),
        "w1": bf(w1), "w1yx": bf(w1yx), "b1": b1sb,
        "w2": bf(w2), "b2": b2sb, "w3": bf(W3), "b3": b3sb,
        "selS": selS, "selB": bf16(selB), "selE": bf16(selE), "selR": bf16(selR),
    }


def kernel(**inputs):
    feat = np.asarray(inputs["feat"], np.float32)
    lr_guide = np.asarray(inputs["lr_guide"], np.float32)
    hr_guide = np.asarray(inputs["hr_guide"], np.float32)
    W1 = np.asarray(inputs["W1"], np.float32)
    b1 = np.asarray(inputs["b1"], np.float32)
    W2 = np.asarray(inputs["W2"], np.float32)
    b2 = np.asarray(inputs["b2"], np.float32)
    W3 = np.asarray(inputs["W3"], np.float32)
    b3 = np.asarray(inputs["b3"], np.float32)

    nc = _build_nc()
    in_maps = [_prep_core(c, feat, lr_guide, hr_guide, W1, b1, W2, b2, W3, b3)
               for c in range(NCORES)]
    res = run_bass_kernel_spmd(nc, in_maps, core_ids=list(range(NCORES)))
    out = np.zeros((1, 32, 256, 256), np.float32)
    for c in range(NCORES):
        yc = np.asarray(res.results[c]["y"]).astype(np.float32)
        yc = yc.reshape(32, 4, 16, 128)
        strip = out[0, :, 32 * c:32 * c + 32, :]
        for p in range(2):
            for q in range(2):
                strip[:, p::2, q::2] = yc[:, 2 * p + q]
    return out
